# revision 1
# baseline (speedup 1.0000x reference)
"""CpxRBM translation-invariant log-psi kernel for 8 Trainium2 NeuronCores.

Computes sum(log(cosh(sym @ W.T))) where sym is the (4095, 4096) matrix of
circular shifts of v = 2*vis_states - 1 and W is (1024, 4096) complex64.

Strategy (shift-sharded, 512 shifts/core; core 7's extra shift row is masked
to zero, which contributes exactly 0 to both accumulated sums):
  - symT chunks are built ON DEVICE from a 4608-element window of the doubled
    v vector via overlapping-stride DMAs (symT[i,s] = vwin[i+s]), one DMA per
    128-row k-chunk so matmuls start almost immediately.
  - Complex matmul: sym is real, so pre = [sym @ Wr.T | sym @ Wi.T].  Host
    interleaves Wr/Wi into one (4096, 2, 1024) bf16 tensor; each (k-chunk,
    o-quarter) is one 128KB DMA and one N=512 matmul per s-tile (the moving
    operand carries both real and imag columns), fp32 PSUM accumulation.
  - log(cosh(x+iy)) elementwise: a = 2cosh(x)cos(y), b = 2sinh(x)sin(y),
      Re = 0.5*ln(a^2+b^2) - ln2
      Im = 2*atan(b / (sqrt(a^2+b^2) + a))        (exact principal atan2)
    sqrt and 1/x both via Exp/Ln so only two ACT table sets are used
    (natural_log_exp_and_others, trig_and_small); an activation-table filter
    plus explicit ordering deps keep it to 2 table loads per o-quarter.
  - Per-core output: (128, 8) fp32 partial sums; host reduces.
"""
import math
import numpy as np
import ml_dtypes
from contextlib import ExitStack

import concourse.bass as bass
import concourse.mybir as mybir
import concourse.tile as tile
from concourse import bacc
from concourse.bass_utils import run_bass_kernel_spmd
from concourse.hw_specs import get_activation_tables
import bass_rust as _bass_rust

F32 = mybir.dt.float32
BF16 = mybir.dt.bfloat16
AF = mybir.ActivationFunctionType
ALU = mybir.AluOpType

PI = float(np.pi)
VIS_N = 4096
INP_N = 4096
OUP_N = 1024
N_CORES = 8
S_PER_CORE = 512
N_KCHUNK = 32
N_QUARTER = 4
OQ = OUP_N // N_QUARTER   # 256
WIN = S_PER_CORE + INP_N  # 4608
N_BLOCKS = N_QUARTER

# Only these ACT table sets may be chosen: exp+ln live together, sin+arctan
# live together -> no table thrash between Ln and Exp or Sin and Arctan.
_ALLOWED_SETS = {"natural_log_exp_and_others", "trig_and_small"}


class _Bacc(bacc.Bacc):
    def insert_act_table_loads(self):
        has_activation = any(
            isinstance(i, mybir.InstActivation)
            for b in self.main_func.blocks
            for i in b.instructions
        )
        if not has_activation:
            return
        tables = [
            (name, funcs if name in _ALLOWED_SETS else set())
            for name, funcs in get_activation_tables(self.m.arch).items()
        ]
        _bass_rust.insert_act_table_loads(self, tables)


_nc_cache = None
last_results = None


def _build_nc():
    nc = _Bacc("TRN2", target_bir_lowering=False, debug=False)

    vwin = nc.dram_tensor("vwin", [WIN], BF16, kind="ExternalInput")
    wc = nc.dram_tensor("wc", [INP_N, 2, OUP_N], BF16, kind="ExternalInput")
    msk = nc.dram_tensor("msk", [128, 1], F32, kind="ExternalInput")
    acc = nc.dram_tensor("acc", [128, 2 * N_BLOCKS], F32, kind="ExternalOutput")

    with tile.TileContext(nc) as tc, ExitStack() as ctx:
        singles = ctx.enter_context(tc.tile_pool(name="singles", bufs=1))
        sympool = ctx.enter_context(tc.tile_pool(name="sympool", bufs=1))
        wpool = ctx.enter_context(tc.tile_pool(name="wpool", bufs=10))
        ppool = ctx.enter_context(tc.tile_pool(name="ppool", bufs=2, space="PSUM"))
        stage = ctx.enter_context(tc.tile_pool(name="stage", bufs=3))
        dpool = ctx.enter_context(tc.tile_pool(name="dpool", bufs=1, space="DRAM"))

        half_pi = singles.tile([128, 1], F32)
        nc.vector.memset(half_pi, PI / 2.0)
        msk_sb = singles.tile([128, 1], F32)
        nc.sync.dma_start(out=msk_sb, in_=msk[:, :])
        acc_sb = singles.tile([128, 2 * N_BLOCKS], F32)

        # symT_c[p, s] = vwin[c*128 + p + s]; one tile per k-chunk so the
        # dependency tracking is exact and matmuls start as chunks land.
        # Interleave sym-window and first-quarter weight DMAs on the two
        # HWDGE queues (sync/scalar, crossed) so chunk c's operands both
        # arrive at ~0.65us*c.  {0,1} -> {-1,+1} per chunk on the vector
        # engine, which is idle at the start.
        symT = []
        w_q0 = []
        for c in range(N_KCHUNK):
            st_c = sympool.tile([128, S_PER_CORE], BF16, tag=f"sym{c}", name=f"sym{c}")
            (nc.sync if c % 2 == 0 else nc.scalar).dma_start(
                out=st_c, in_=bass.AP(vwin, c * 128, [[1, 128], [1, S_PER_CORE]])
            )
            nc.vector.tensor_scalar(st_c, st_c, 2.0, 1.0, ALU.mult, ALU.subtract)
            # core 7 zeroes the 512th shift's column (a zero sym row
            # contributes exactly 0 to both accumulated sums)
            nc.vector.tensor_scalar(
                st_c[:, S_PER_CORE - 1 : S_PER_CORE],
                st_c[:, S_PER_CORE - 1 : S_PER_CORE],
                msk_sb, None, ALU.mult,
            )
            symT.append(st_c)
            w_t = wpool.tile([128, 2, OQ], BF16, tag=f"wq0_{c}", name=f"wq0_{c}", bufs=1)
            nc.gpsimd.dma_start(out=w_t, in_=wc[c * 128 : (c + 1) * 128, :, 0:OQ])
            w_q0.append(w_t)

        state = {"prev_at": None}

        def emit_elementwise(pxr, pxi, nst, blk, last=False):
            """log(cosh) on the given psum slices ((128, nst, OQ) each),
            accumulating into acc_sb columns (2*blk, 2*blk+1).  For final
            blocks, read x straight from psum (no later user of the banks)."""
            ow = OQ
            g = stage.tile([128, nst, ow], F32, tag="g")
            l = stage.tile([128, nst, ow], F32, tag="l")
            u = stage.tile([128, nst, ow], F32, tag="u")
            sy = stage.tile([128, nst, ow], F32, tag="sy")
            cy = stage.tile([128, nst, ow], F32, tag="cy")
            ep = stage.tile([128, nst, ow], F32, tag="ep")
            em = stage.tile([128, nst, ow], F32, tag="em")

            if last:
                xr = pxr
                xi = pxi
            else:
                # copy out promptly (on the scalar engine, which has slack)
                # so the psum banks free up for the next quarter
                xr = stage.tile([128, nst, ow], F32, tag="xr")
                xi = stage.tile([128, nst, ow], F32, tag="xi")
                nc.scalar.copy(xr, pxr)
                nc.scalar.copy(xi, pxi)

            # range-reduce y into [-pi, pi]
            nc.vector.tensor_scalar(g, xi, PI, 2.0 * PI, ALU.is_gt, ALU.mult)
            nc.vector.tensor_scalar(l, xi, -PI, 2.0 * PI, ALU.is_lt, ALU.mult)
            nc.vector.scalar_tensor_tensor(u, g, -1.0, xi, ALU.mult, ALU.add)
            nc.vector.tensor_tensor(u, u, l, ALU.add)            # u
            # |u| for the cosine:  cos(y) = sin(pi/2 - |u|), arg in [-pi/2, pi/2]
            nc.vector.scalar_tensor_tensor(l, u, -1.0, u, ALU.mult, ALU.max)

            x2p = stage.tile([128, nst, ow], F32, tag="x2p", bufs=1)
            x2m = stage.tile([128, nst, ow], F32, tag="x2m", bufs=1)

            i_sy = nc.scalar.activation(sy, u, AF.Sin)                # sin(y)
            i_cy = nc.scalar.activation(cy, l, AF.Sin, bias=half_pi, scale=-1.0)
            i_ep = nc.scalar.activation(ep, xr, AF.Exp)               # e^x
            i_em = nc.scalar.activation(em, xr, AF.Exp, scale=-1.0)   # e^-x
            i_2p = nc.scalar.activation(x2p, xr, AF.Exp, scale=2.0)   # e^2x
            i_2m = nc.scalar.activation(x2m, xr, AF.Exp, scale=-2.0)  # e^-2x
            exps = (i_ep, i_em, i_2p, i_2m)
            trigs = (i_sy, i_cy)
            # ACT table-set ordering.  Mid-phase: [trig] -> [exp/ln] -> atan
            # (2 loads/quarter).  Last quarter: exp-block FIRST so it fills
            # the ACT-idle window while the DVE range-reduce runs; costs two
            # extra table loads but pulls the Ln chain several us earlier.
            first, second = (exps, trigs) if last else (trigs, exps)
            for a in second:
                for b in first:
                    tile.add_dep_helper(a.ins, b.ins, reason="act-set order")
            if state["prev_at"] is not None:
                for b in first:
                    tile.add_dep_helper(b.ins, state["prev_at"].ins, reason="act order")

            # |2cosh z|^2 = e^2x + e^-2x + 2 - 4 sin^2 y  -- short path to Ln
            nc.vector.tensor_tensor(l, sy, sy, ALU.mult)          # sin^2 y
            nc.vector.scalar_tensor_tensor(u, x2p, 2.0, x2m, ALU.add, ALU.add)
            nc.vector.scalar_tensor_tensor(x2p, l, -4.0, u, ALU.mult, ALU.add)
            # near-cancellation can round to <= 0; clamp keeps Ln finite
            nc.vector.tensor_scalar(x2p, x2p, 1e-12, None, ALU.max)
            nc.scalar.activation(
                g, x2p, AF.Ln, accum_out=acc_sb[:, 2 * blk : 2 * blk + 1]
            )
            nc.scalar.activation(x2m, g, AF.Exp, scale=0.5)       # r = sqrt(q)
            # imag operands (only needed after r: overlaps the Ln/Exp above)
            nc.vector.tensor_tensor(u, ep, em, ALU.add)           # t1 = 2cosh x
            nc.vector.tensor_tensor(l, ep, em, ALU.subtract)      # t2 = 2sinh x
            nc.vector.tensor_tensor(ep, u, cy, ALU.mult)          # a
            nc.vector.tensor_tensor(u, l, sy, ALU.mult)           # b
            nc.vector.tensor_tensor(em, x2m, ep, ALU.add)         # den = r + a
            # near the branch cut fp32 rounding can push den <= 0; clamp so Ln
            # stays finite (t then blows up -> atan -> +-pi/2, correct limit).
            nc.vector.tensor_scalar(em, em, 1e-20, None, ALU.max)
            nc.scalar.activation(cy, em, AF.Ln)
            nc.scalar.activation(l, cy, AF.Exp, scale=-1.0)       # 1/den
            nc.vector.tensor_tensor(sy, u, l, ALU.mult)           # t = b/den
            state["prev_at"] = nc.scalar.activation(
                cy, sy, AF.Arctan, accum_out=acc_sb[:, 2 * blk + 1 : 2 * blk + 2]
            )

        for q in range(N_QUARTER):
            ps = ppool.tile([128, 4, 2, OQ], F32, tag="ps")
            for c in range(N_KCHUNK):
                if q == 0:
                    w_t = w_q0[c]
                else:
                    w_t = wpool.tile([128, 2, OQ], BF16, tag="w")
                    eng = nc.sync if c % 2 == 0 else nc.scalar
                    eng.dma_start(
                        out=w_t,
                        in_=wc[c * 128 : (c + 1) * 128, :, q * OQ : (q + 1) * OQ],
                    )
                for st in range(4):
                    nc.tensor.matmul(
                        ps[:, st, :, :],
                        symT[c][:, st * 128 : (st + 1) * 128],
                        w_t[:, :, :],
                        start=(c == 0), stop=(c == N_KCHUNK - 1),
                    )

            emit_elementwise(
                ps[:, :, 0, :], ps[:, :, 1, :], 4, q, last=(q == N_QUARTER - 1)
            )

        nc.sync.dma_start(out=acc[:, :], in_=acc_sb)

    nc.finalize()
    return nc


def _get_nc():
    global _nc_cache
    if _nc_cache is None:
        _nc_cache = _build_nc()
    return _nc_cache


def kernel(vis_states: np.ndarray, weights: np.ndarray) -> np.ndarray:
    global last_results
    vis = np.asarray(vis_states).astype(np.float32)
    vv = np.concatenate([vis, vis]).astype(ml_dtypes.bfloat16)  # {0,1}, exact
    w = np.asarray(weights)
    wc = np.empty((INP_N, 2, OUP_N), dtype=ml_dtypes.bfloat16)
    wc[:, 0, :] = w.real.astype(np.float32).T
    wc[:, 1, :] = w.imag.astype(np.float32).T

    in_maps = []
    for c in range(N_CORES):
        s0 = c * S_PER_CORE
        m = np.ones((128, 1), np.float32)
        if c == N_CORES - 1:
            m[:] = 0.0  # zero the sym column of the nonexistent 4096th shift
        in_maps.append(
            {"vwin": np.ascontiguousarray(vv[s0 : s0 + WIN]), "wc": wc, "msk": m}
        )

    nc = _get_nc()
    res = run_bass_kernel_spmd(nc, in_maps, core_ids=list(range(N_CORES)))
    last_results = res

    tot_ln = 0.0
    tot_at = 0.0
    for r in res.results:
        a = r["acc"].astype(np.float64)
        tot_ln += a[:, 0::2].sum()
        tot_at += a[:, 1::2].sum()

    n_counted = N_CORES * S_PER_CORE * OUP_N  # includes the masked zero row
    real = 0.5 * tot_ln - math.log(2.0) * n_counted
    imag = 2.0 * tot_at
    return np.array(real + 1j * imag, dtype=np.complex64)



# revision 6
# speedup vs baseline: 1.1222x; 1.1222x over previous
"""CpxRBM translation-invariant log-psi kernel for 8 Trainium2 NeuronCores.

Computes sum(log(cosh(sym @ W.T))) where sym is the (4095, 4096) matrix of
circular shifts of v = 2*vis_states - 1 and W is (1024, 4096) complex64.

v3 strategy (shift-sharded, 512 shifts/core, fp8 DoubleRow matmuls):
  - Host sends the per-core 4608-window of doubled v as fp8 {-1,+1} (exact)
    and the weights as fp8e4 scaled by 64 (power of two; undone for free via
    ACT affine scales).  Core 7's phantom 4096th shift (the wrap-around
    shift 4095) is subtracted EXACTLY on the host using the same
    fp8-dequantized weights.
  - Matmul: perf_mode=DoubleRow packs 2 fp8 weights/PE cell: each matmul
    contracts K=256 (two 128-row chunks) with a 128x1024-fp8 moving operand
    -> 256 matmuls x 512 out cols instead of 512 (bf16): ~2x tensor time.
    sym slices are stationary; weights move; fp32 PSUM accumulation.
  - Loop nest is OUTPUT-QUARTER-outer so each 13.8us matmul phase consumes
    only 2.1MB of weights (~152GB/s) - the weight stream pipelines instead
    of starving phase 0 (HBM is ~358GB/s).
  - log(cosh(x+iy)) elementwise, 9 ACT passes/block:
      half-angle trig (no range reduction; |y| <= ~3.6 and the HW sin table
      is accurate past pi - verified by probe):
        sh = sin(y/2), ch = sin(pi/2 - y/2), p = sh*ch = sin(y)/2,
        cy = 1 - 2 sh^2 = cos y
      q = |2cosh z|^2 = (e^x + e^-x)^2 - 16 p^2
      Re = 0.5*ln(q + 1e-6) - ln2      (ACT Ln accumulates row sums; the
                                        1e-6 ACT bias replaces a clamp)
      Im = 2*atan(2*b' / (sqrt(q) + a + 1e-4)),  a = t1*cy, b' = t2*p
        (HW arctan verified accurate over the full input range)
      sqrt/recip via Exp/Ln so only two ACT table sets are used; the
      Arctan of block i rides in block i+1's trig-table residency.
  - ACT reads x,y straight from PSUM (frees banks after the Exp passes).
  - Per-core output: (128, 8) fp32 accumulator columns; host reduces.
"""
import math
import numpy as np
import ml_dtypes
from contextlib import ExitStack

import concourse.bass as bass
import concourse.mybir as mybir
import concourse.tile as tile
from concourse import bacc
from concourse.bass_utils import run_bass_kernel_spmd
from concourse.hw_specs import get_activation_tables
import bass_rust as _bass_rust

F32 = mybir.dt.float32
BF16 = mybir.dt.bfloat16
FP8 = mybir.dt.float8e4
AF = mybir.ActivationFunctionType
ALU = mybir.AluOpType
DR = mybir.MatmulPerfMode.DoubleRow

PI = float(np.pi)
VIS_N = 4096
INP_N = 4096
OUP_N = 1024
N_CORES = 8
S_PER_CORE = 512
N_K2 = 16                  # 256-row contraction chunks
N_ST = 4                   # shift tiles of 128 per core
N_Q = 4                    # output quarters (phases)
OQ = OUP_N // N_Q          # 256 output cols per quarter
WIN = S_PER_CORE + INP_N   # 4608
SCALE = 64.0               # fp8 weight scale (power of 2)
EPS_Q = 1e-6               # Ln bias: absorbs fp32 rounding of q ~ 0
EPS_D = 1e-4               # Ln bias: absorbs fp32 rounding of den ~ 0

_ALLOWED_SETS = {"natural_log_exp_and_others", "trig_and_small"}


class _Bacc(bacc.Bacc):
    def insert_act_table_loads(self):
        has_activation = any(
            isinstance(i, mybir.InstActivation)
            for b in self.main_func.blocks
            for i in b.instructions
        )
        if not has_activation:
            return
        tables = [
            (name, funcs if name in _ALLOWED_SETS else set())
            for name, funcs in get_activation_tables(self.m.arch).items()
        ]
        _bass_rust.insert_act_table_loads(self, tables)


_nc_cache = None
last_results = None


def _build_nc():
    nc = _Bacc("TRN2", target_bir_lowering=False, debug=False)

    vwin = nc.dram_tensor("vwin", [WIN], FP8, kind="ExternalInput")
    # weights, quarter-major: wq[q*INP_N + k, j*OQ + o] fp8
    wq = nc.dram_tensor("wq", [N_Q * INP_N, 2 * OQ], FP8, kind="ExternalInput")
    acc = nc.dram_tensor("acc", [128, 2 * N_Q], F32, kind="ExternalOutput")

    with tile.TileContext(nc) as tc, ExitStack() as ctx:
        singles = ctx.enter_context(tc.tile_pool(name="singles", bufs=1))
        sympool = ctx.enter_context(tc.tile_pool(name="sympool", bufs=1))
        wpool = ctx.enter_context(tc.tile_pool(name="wpool", bufs=1))
        ppool = ctx.enter_context(tc.tile_pool(name="ppool", bufs=2, space="PSUM"))
        stage = ctx.enter_context(tc.tile_pool(name="stage", bufs=2))

        acc_sb = singles.tile([128, 2 * N_Q], F32)
        half_pi = singles.tile([128, 1], F32)
        nc.vector.memset(half_pi, PI / 2.0)
        eps_q = singles.tile([128, 1], F32)
        nc.vector.memset(eps_q, EPS_Q)
        eps_d = singles.tile([128, 1], F32)
        nc.vector.memset(eps_d, EPS_D)

        # sym windows: symt[k2][p, i, s] = vwin[256*k2 + 128*i + p + s]
        symt = []
        for k2 in range(N_K2):
            st_t = sympool.tile([128, 2, S_PER_CORE], FP8, tag=f"sym{k2}",
                                name=f"sym{k2}")
            nc.scalar.dma_start(
                out=st_t,
                in_=bass.AP(vwin, 256 * k2, [[1, 128], [128, 2], [1, S_PER_CORE]]),
            )
            symt.append(st_t)
        # weights: wt[q][k2][p, i, jo] = wq[q*4096 + 256*k2 + 128*i + p, jo]
        # 131KB each, emitted in consumption order on the sync HWDGE ring.
        wt = [[None] * N_K2 for _ in range(N_Q)]
        for q in range(N_Q):
            for k2 in range(N_K2):
                w_t = wpool.tile([128, 2, 2 * OQ], FP8, tag=f"w{q}_{k2}",
                                 name=f"w{q}_{k2}")
                nc.sync.dma_start(
                    out=w_t,
                    in_=bass.AP(
                        wq, (q * INP_N + 256 * k2) * 512,
                        [[512, 128], [128 * 512, 2], [1, 512]],
                    ),
                )
                wt[q][k2] = w_t

        state = {"prev_t": None, "prev_col": None, "prev_ops": []}

        def emit_block(blk, ps, last=False):
            """log(cosh) on ps ([128, 4, 512] fp32, free cols = [re | im]
            halves of 256), accumulating into acc_sb cols (2b, 2b+1)."""
            xr = ps[:, :, 0:OQ]
            xi = ps[:, :, OQ:2 * OQ]
            shp = [128, 4, OQ]

            A = stage.tile(shp, F32, tag="A")
            B = stage.tile(shp, F32, tag="B")
            C = stage.tile(shp, F32, tag="C")
            D = stage.tile(shp, F32, tag="D")
            E = stage.tile(shp, F32, tag="E")
            F = stage.tile(shp, F32, tag="F")
            G = stage.tile(shp, F32, tag="G")
            R1 = stage.tile(shp, BF16, tag="R1")
            R3 = stage.tile(shp, BF16, tag="R3")
            R4 = stage.tile(shp, BF16, tag="R4")

            # ACT trig phase; the previous block's Arctan rides along.
            i_sh = nc.scalar.activation(A, xi, AF.Sin, scale=0.5 / SCALE)
            i_ch = nc.scalar.activation(B, xi, AF.Sin, bias=half_pi,
                                        scale=-0.5 / SCALE)
            trig_ops = [i_sh, i_ch]
            if state["prev_t"] is not None:
                i_at = nc.scalar.activation(
                    R4, state["prev_t"], AF.Arctan, scale=2.0,
                    accum_out=acc_sb[:, state["prev_col"]:state["prev_col"] + 1],
                )
                trig_ops.append(i_at)
            # ACT exp phase
            i_ep = nc.scalar.activation(C, xr, AF.Exp, scale=1.0 / SCALE)
            i_em = nc.scalar.activation(D, xr, AF.Exp, scale=-1.0 / SCALE)
            for a_op in (i_ep, i_em):
                for b_op in trig_ops:
                    tile.add_dep_helper(a_op.ins, b_op.ins, reason="act-set order")
            for b_op in trig_ops:
                for pr in state["prev_ops"]:
                    tile.add_dep_helper(b_op.ins, pr.ins, reason="act order")

            # DVE chains (fp32 until the final bf16 products)
            nc.vector.tensor_tensor(E, A, A, ALU.mult)            # sh^2
            nc.vector.tensor_scalar(E, E, -2.0, 1.0, ALU.mult, ALU.add)  # cy
            nc.vector.tensor_tensor(B, A, B, ALU.mult)            # p = sh*ch
            nc.vector.tensor_tensor(A, B, B, ALU.mult)            # p^2
            nc.vector.tensor_tensor(F, C, D, ALU.add)             # t1 = 2cosh
            nc.vector.tensor_tensor(C, C, D, ALU.subtract)        # t2 = 2sinh
            nc.vector.tensor_tensor(D, F, F, ALU.mult)            # t1^2
            nc.vector.scalar_tensor_tensor(A, A, -16.0, D, ALU.mult, ALU.add)
            i_lnq = nc.scalar.activation(
                D, A, AF.Ln, bias=eps_q,
                accum_out=acc_sb[:, 2 * blk:2 * blk + 1],
            )
            i_r = nc.scalar.activation(G, D, AF.Exp, scale=0.5)   # r = sqrt(q)
            nc.vector.tensor_tensor(E, F, E, ALU.mult)            # a = t1*cy
            nc.vector.tensor_tensor(R3, C, B, ALU.mult)           # b' = t2*p
            nc.vector.tensor_tensor(G, G, E, ALU.add)             # den = r + a
            i_lnd = nc.scalar.activation(E, G, AF.Ln, bias=eps_d)
            i_inv = nc.scalar.activation(R1, E, AF.Exp, scale=-1.0)
            nc.vector.tensor_tensor(R3, R3, R1, ALU.mult)         # t = b'/den
            state["prev_t"] = R3
            state["prev_col"] = 2 * blk + 1
            state["prev_ops"] = [i_ep, i_em, i_lnq, i_r, i_lnd, i_inv]

            if last:
                i_at = nc.scalar.activation(
                    R4, R3, AF.Arctan, scale=2.0,
                    accum_out=acc_sb[:, 2 * blk + 1:2 * blk + 2],
                )
                for pr in state["prev_ops"]:
                    tile.add_dep_helper(i_at.ins, pr.ins, reason="act order")
                state["prev_t"] = None

        for q in range(N_Q):
            ps = ppool.tile([128, N_ST, 2 * OQ], F32, tag="ps")
            for st in range(N_ST):
                for k2 in range(N_K2):
                    nc.tensor.matmul(
                        ps[:, st, :],
                        symt[k2][:, :, st * 128:(st + 1) * 128],
                        wt[q][k2][:, :, :],
                        start=(k2 == 0), stop=(k2 == N_K2 - 1),
                        perf_mode=DR,
                    )
            emit_block(q, ps, last=(q == N_Q - 1))

        nc.sync.dma_start(out=acc[:, :], in_=acc_sb)

    nc.finalize()
    return nc


def _get_nc():
    global _nc_cache
    if _nc_cache is None:
        _nc_cache = _build_nc()
    return _nc_cache


def _host_prep(vis_states, weights):
    vis = np.asarray(vis_states).astype(np.float32)
    v = 2.0 * vis - 1.0
    vv = np.concatenate([v, v]).astype(ml_dtypes.float8_e4m3)  # +-1, exact
    w = np.asarray(weights)
    ws_r = (w.real.astype(np.float32).T * np.float32(SCALE)).astype(
        ml_dtypes.float8_e4m3)                                  # (4096, 1024)
    ws_i = (w.imag.astype(np.float32).T * np.float32(SCALE)).astype(
        ml_dtypes.float8_e4m3)
    # quarter-major: wq[q*4096 + k, j*256 + o] = ws_{r,i}[k, q*256 + o]
    wq = np.empty((N_Q, INP_N, 2, OQ), dtype=ml_dtypes.float8_e4m3)
    wq[:, :, 0, :] = np.moveaxis(ws_r.reshape(INP_N, N_Q, OQ), 1, 0)
    wq[:, :, 1, :] = np.moveaxis(ws_i.reshape(INP_N, N_Q, OQ), 1, 0)
    return v, vv, ws_r, ws_i, wq.reshape(N_Q * INP_N, 2 * OQ)


def kernel(vis_states: np.ndarray, weights: np.ndarray) -> np.ndarray:
    global last_results
    v, vv, ws_r, ws_i, wq = _host_prep(vis_states, weights)

    in_maps = []
    for c in range(N_CORES):
        s0 = c * S_PER_CORE
        in_maps.append(
            {"vwin": np.ascontiguousarray(vv[s0:s0 + WIN]), "wq": wq}
        )

    nc = _get_nc()
    res = run_bass_kernel_spmd(nc, in_maps, core_ids=list(range(N_CORES)))
    last_results = res

    tot_ln = 0.0
    tot_at = 0.0
    for r in res.results:
        a = r["acc"].astype(np.float64)
        tot_ln += a[:, 0::2].sum()
        tot_at += a[:, 1::2].sum()

    n_counted = N_CORES * S_PER_CORE * OUP_N  # includes the phantom shift
    real = 0.5 * tot_ln - math.log(2.0) * n_counted
    imag = 2.0 * tot_at

    # subtract the phantom wrap-around shift 4095 (core 7 row 512), using
    # the SAME fp8-dequantized weights the device used.
    v4095 = np.concatenate([v, v])[4095:4095 + INP_N].astype(np.float64)
    pre_r = v4095 @ ws_r.astype(np.float64) / SCALE          # (1024,)
    pre_i = v4095 @ ws_i.astype(np.float64) / SCALE
    phantom = np.log(np.cosh(pre_r + 1j * pre_i)).sum()
    real -= phantom.real
    imag -= phantom.imag

    return np.array(real + 1j * imag, dtype=np.complex64)
